# revision 8
# baseline (speedup 1.0000x reference)
"""Capsule routing pooling kernel for Trainium2 (8 NeuronCores, data parallel).

Math: the reference's softmax is over a singleton axis, so the routing
coefficients are identically 1.0 and the routing iterations never affect the
output.  The computation reduces to, per (b, c, 2x2 spatial tile):
    s   = sum of the four D=16 vectors in the tile
    sq  = sum_d s_d^2
    out = s * sq / ((1 + sq) * (sqrt(sq) + 1e-8)) = s * sqrt(sq) / (1 + sq)

Sharding: batch dim (16) split across 8 cores -> 2 batches/core.  Per core the
(2*64)=128 (b,c) pairs map onto the 128 SBUF partitions; each partition owns a
full 64x64x16 image.

v5: DMA-read-roofline design (~33.5MB f32 reads/core @ ~400GB/s measured).
  - all steady-state loads cast f32->bf16 in the DMA datapath (SWDGE) on ONE
    queue, so arrivals are FIFO in consumption order (two load queues invert:
    packet-round-robin starves the older queue and the in-order DVE stream
    stalls on it).  Only a short first super-group loads f32 via HWDGE, which
    boots ~5us before the Q7 SWDGE path; it drains before Q0 ramps.
  - every DVE elementwise op runs in bf16 2x packed mode; the D=16 reduction
    is a pairwise tree of bf16 adds (2x) instead of tensor_reduce (1x).
  - squares run on the otherwise-idle ACT engine, emitted with their
    super-group's front so the cross-engine hop hides under the next SG's
    adds (tails one SG late); stores issue from the idle sync engine so they
    never sit in front of ACT work.
  - scale = sqrt(sq)/(1+sq); reciprocal runs before the sqrt hop returns.
"""

import numpy as np

import concourse.bass as bass
import concourse.bacc as bacc
import concourse.tile as tile
from concourse import mybir
from concourse.bass_utils import run_bass_kernel_spmd

_B, _C, _H, _W, _D = 16, 64, 64, 64, 16
_NCORES = 8
_F32 = mybir.dt.float32
_BF16 = mybir.dt.bfloat16


def _kernel_body(tc, out_ap, in_ap, H, W, D):
    nc = tc.nc
    P = 128
    nH, nW = H // 2, W // 2

    inv4 = in_ap.rearrange("p (q four) w d -> p q (four w d)", four=4)
    inv2 = in_ap.rearrange("p (rp two) w d -> p rp (two w d)", two=2)
    outv = out_ap.rearrange("p y x d -> p y (x d)")

    if nH == 32:
        # SG0=2 (fine, HWDGE f32: boots ~5us before Q7 and fully drains
        # before Q0's first SWDGE transfer so the queues never round-robin)
        sched = [2, 4, 4, 4, 4, 4, 4, 4, 1, 1]
    elif nH % 4 == 0:
        sched = [4] * (nH // 4)
    else:
        sched = [nH]
    assert sum(sched) == nH

    import contextlib

    with contextlib.ExitStack() as ctx:
        slabs = ctx.enter_context(tc.tile_pool(name="slabs", bufs=12))
        rpool = ctx.enter_context(tc.tile_pool(name="rpool", bufs=3))
        mid = ctx.enter_context(tc.tile_pool(name="mid", bufs=3))
        sqp = ctx.enter_context(tc.tile_pool(name="sqp", bufs=3))
        tree = ctx.enter_context(tc.tile_pool(name="tree", bufs=2))
        small = ctx.enter_context(tc.tile_pool(name="small", bufs=2))
        outp = ctx.enter_context(tc.tile_pool(name="outp", bufs=3))

        def emit_front(sg, g0, fine=False, hwdge=False):
            """loads + row-pair adds + col-pair adds + ACT square for one
            super-group of `sg` row-pairs starting at output row g0."""
            s_sg = mid.tile([P, sg, nW, D], _BF16, tag="s_sg")
            ci = 0
            while ci < sg:
                if fine or sg - ci < 2:
                    rp = g0 + ci
                    sdt = _F32 if hwdge else _BF16
                    slab = slabs.tile([P, 1, 2, nW, 2, D], sdt, tag="slab")
                    eng = nc.sync if hwdge else nc.gpsimd
                    eng.dma_start(
                        out=slab[:],
                        in_=inv2[:, rp, :].rearrange("p (two b) -> p two b", two=2),
                    )
                    r = rpool.tile([P, 1, nW, 2, D], _BF16, tag="r")
                    nc.vector.tensor_add(
                        r[:], slab[:, :, 0, :, :, :], slab[:, :, 1, :, :, :]
                    )
                    nc.vector.tensor_add(
                        s_sg[:, ci : ci + 1, :, :], r[:, :, :, 0, :], r[:, :, :, 1, :]
                    )
                    ci += 1
                else:
                    t = (g0 + ci) // 2
                    sdt = _F32 if hwdge else _BF16
                    slab = slabs.tile([P, 2, 2, nW, 2, D], sdt, tag="slab")
                    eng = nc.sync if hwdge else nc.gpsimd
                    eng.dma_start(
                        out=slab[:],
                        in_=inv4[:, t, :].rearrange(
                            "p (a two b) -> p a two b", a=2, two=2
                        ),
                    )
                    r = rpool.tile([P, 2, nW, 2, D], _BF16, tag="r")
                    nc.vector.tensor_add(
                        r[:], slab[:, :, 0, :, :, :], slab[:, :, 1, :, :, :]
                    )
                    nc.vector.tensor_add(
                        s_sg[:, ci : ci + 2, :, :], r[:, :, :, 0, :], r[:, :, :, 1, :]
                    )
                    ci += 2
            # square on ACT now: its input is ready and it hides under the
            # next SG's adds on DVE
            nsg = sg * nW
            sv = s_sg[:].rearrange("p s x d -> p (s x) d")
            sq2 = sqp.tile([P, nsg, D], _BF16, tag="sq2")
            nc.scalar.activation(sq2[:], sv, mybir.ActivationFunctionType.Square)
            return s_sg, sq2

        def emit_rest(sg, g0, s_sg, sq2):
            """tree-reduce over D + scale chain + final multiply + store."""
            nsg = sg * nW
            sv = s_sg[:].rearrange("p s x d -> p (s x) d")
            t8 = tree.tile([P, nsg, 8], _BF16, tag="t8")
            nc.vector.tensor_add(t8[:], sq2[:, :, 0:8], sq2[:, :, 8:16])
            t4 = tree.tile([P, nsg, 4], _BF16, tag="t4")
            nc.vector.tensor_add(t4[:], t8[:, :, 0:4], t8[:, :, 4:8])
            t2 = tree.tile([P, nsg, 2], _BF16, tag="t2")
            nc.vector.tensor_add(t2[:], t4[:, :, 0:2], t4[:, :, 2:4])
            ch = small.tile([P, nsg, 5], _F32, tag="ch")
            sq = ch[:, :, 0:1]
            c1 = ch[:, :, 1:2]
            a = ch[:, :, 2:3]
            rec = ch[:, :, 3:4]
            sc = ch[:, :, 4:5]
            nc.vector.tensor_add(sq, t2[:, :, 0:1], t2[:, :, 1:2])
            # scale = sqrt(sq) / (1 + sq)   (1e-8 dropped: sq >= O(1))
            nc.vector.tensor_scalar_add(c1, sq, 1.0)
            nc.scalar.activation(a, sq, mybir.ActivationFunctionType.Sqrt)
            nc.vector.reciprocal_approx_fast(rec, c1)
            nc.vector.tensor_mul(sc, a, rec)
            outt = outp.tile([P, sg, nW, D], _F32, tag="outt")
            ov = outt[:].rearrange("p s x d -> p (s x) d")
            nc.vector.tensor_mul(ov, sv, sc.to_broadcast((P, nsg, D)))
            nc.sync.dma_start(
                out=outv[:, g0 : g0 + sg, :],
                in_=ov.rearrange("p n d -> p (n d)"),
            )

        pending = []
        g0 = 0
        for si, sg in enumerate(sched):
            fine = si == 0 or si >= len(sched) - 2
            front = emit_front(sg, g0, fine=fine, hwdge=(si == 0))
            if pending:
                sg_p, g0_p, (s_p, sq_p) = pending.pop(0)
                emit_rest(sg_p, g0_p, s_p, sq_p)
            pending.append((sg, g0, front))
            g0 += sg
        for sg_p, g0_p, (s_p, sq_p) in pending:
            emit_rest(sg_p, g0_p, s_p, sq_p)


def build_nc(H=_H, W=_W, D=_D):
    """Build and compile the per-core Bass program."""
    nc = bacc.Bacc("TRN2", target_bir_lowering=False, debug=False)
    inp = nc.dram_tensor("inp", [128, H, W, D], _F32, kind="ExternalInput").ap()
    out = nc.dram_tensor(
        "out", [128, H // 2, W // 2, D], _F32, kind="ExternalOutput"
    ).ap()
    with tile.TileContext(nc) as tc:
        _kernel_body(tc, out, inp, H, W, D)
    nc.compile()
    return nc


_NC_CACHE = {}


def _get_nc():
    if "nc" not in _NC_CACHE:
        _NC_CACHE["nc"] = build_nc()
    return _NC_CACHE["nc"]


def kernel(inp, kernel_size=2, routing_iteration=3, _trace=False, _tmpdir=None):
    inp = np.asarray(inp, dtype=np.float32)
    assert int(kernel_size) == 2, "kernel compiled for kernel_size=2"
    assert inp.shape == (_B, _C, _H, _W, _D), inp.shape
    # routing_iteration is mathematically irrelevant (softmax over singleton
    # axis -> coefficients identically 1); any value >= 1 gives this output.

    nc = _get_nc()
    bpc = _B // _NCORES  # batches per core
    in_maps = [
        {"inp": np.ascontiguousarray(inp[i * bpc : (i + 1) * bpc]).reshape(128, _H, _W, _D)}
        for i in range(_NCORES)
    ]
    res = run_bass_kernel_spmd(
        nc, in_maps, core_ids=list(range(_NCORES)), trace=_trace, tmpdir=_tmpdir
    )
    out = np.empty((_B, _C, _H // 2, _W // 2, _D), dtype=np.float32)
    for i in range(_NCORES):
        out[i * bpc : (i + 1) * bpc] = res.results[i]["out"].reshape(
            bpc, _C, _H // 2, _W // 2, _D
        )
    if _trace:
        return out, res
    return out


# revision 9
# speedup vs baseline: 1.2037x; 1.2037x over previous
"""Capsule routing pooling kernel for Trainium2 (8 NeuronCores, data parallel).

Math: the reference's softmax is over a singleton axis, so the routing
coefficients are identically 1.0 and the routing iterations never affect the
output.  The computation reduces to, per (b, c, 2x2 spatial tile):
    s   = sum of the four D=16 vectors in the tile
    sq  = sum_d s_d^2
    out = s * sq / ((1 + sq) * (sqrt(sq) + 1e-8)) = s * sqrt(sq) / (1 + sq)

Sharding: batch dim (16) split across 8 cores -> 2 batches/core.  Per core the
(2*64)=128 (b,c) pairs map onto the 128 SBUF partitions; each partition owns a
full 64x64x16 image.

v5: DMA-read-roofline design (~33.5MB f32 reads/core @ ~400GB/s measured).
  - all steady-state loads cast f32->bf16 in the DMA datapath (SWDGE) on ONE
    queue, so arrivals are FIFO in consumption order (two load queues invert:
    packet-round-robin starves the older queue and the in-order DVE stream
    stalls on it).  Only a short first super-group loads f32 via HWDGE, which
    boots ~5us before the Q7 SWDGE path; it drains before Q0 ramps.
  - every DVE elementwise op runs in bf16 2x packed mode; the D=16 reduction
    is a pairwise tree of bf16 adds (2x) instead of tensor_reduce (1x).
  - squares run on the otherwise-idle ACT engine, emitted with their
    super-group's front so the cross-engine hop hides under the next SG's
    adds (tails one SG late); stores issue from the idle sync engine so they
    never sit in front of ACT work.
  - scale = sqrt(sq)/(1+sq); reciprocal runs before the sqrt hop returns.
"""

import numpy as np

import concourse.bass as bass
import concourse.bacc as bacc
import concourse.tile as tile
from concourse import mybir
from concourse.bass_utils import run_bass_kernel_spmd

_B, _C, _H, _W, _D = 16, 64, 64, 64, 16
_NCORES = 8
_F32 = mybir.dt.float32
_BF16 = mybir.dt.bfloat16


def _kernel_body(tc, out_ap, in_ap, H, W, D):
    nc = tc.nc
    P = 128
    nH, nW = H // 2, W // 2

    inv4 = in_ap.rearrange("p (q four) w d -> p q (four w d)", four=4)
    inv2 = in_ap.rearrange("p (rp two) w d -> p rp (two w d)", two=2)
    outv = out_ap.rearrange("p y x d -> p y (x d)")

    if nH == 32:
        # SG0=2 (fine, HWDGE f32: boots ~5us before Q7 and fully drains
        # before Q0's first SWDGE transfer so the queues never round-robin)
        sched = [2, 4, 4, 4, 4, 4, 4, 4, 1, 1]
    elif nH % 4 == 0:
        sched = [4] * (nH // 4)
    else:
        sched = [nH]
    assert sum(sched) == nH

    import contextlib

    with contextlib.ExitStack() as ctx:
        slabs = ctx.enter_context(tc.tile_pool(name="slabs", bufs=16))
        rpool = ctx.enter_context(tc.tile_pool(name="rpool", bufs=3))
        mid = ctx.enter_context(tc.tile_pool(name="mid", bufs=3))
        sqp = ctx.enter_context(tc.tile_pool(name="sqp", bufs=3))
        tree = ctx.enter_context(tc.tile_pool(name="tree", bufs=2))
        small = ctx.enter_context(tc.tile_pool(name="small", bufs=2))
        outp = ctx.enter_context(tc.tile_pool(name="outp", bufs=2))

        def emit_front(sg, g0, fine=False, hwdge=False):
            """loads + row-pair adds + col-pair adds + ACT square for one
            super-group of `sg` row-pairs starting at output row g0."""
            s_sg = mid.tile([P, sg, nW, D], _BF16, tag="s_sg")
            ci = 0
            while ci < sg:
                if fine or sg - ci < 2:
                    rp = g0 + ci
                    sdt = _F32 if hwdge else _BF16
                    slab = slabs.tile([P, 1, 2, nW, 2, D], sdt, tag="slab")
                    eng = nc.sync if hwdge else nc.gpsimd
                    eng.dma_start(
                        out=slab[:],
                        in_=inv2[:, rp, :].rearrange("p (two b) -> p two b", two=2),
                    )
                    r = rpool.tile([P, 1, nW, 2, D], _BF16, tag="r")
                    nc.vector.tensor_add(
                        r[:], slab[:, :, 0, :, :, :], slab[:, :, 1, :, :, :]
                    )
                    nc.vector.tensor_add(
                        s_sg[:, ci : ci + 1, :, :], r[:, :, :, 0, :], r[:, :, :, 1, :]
                    )
                    ci += 1
                else:
                    t = (g0 + ci) // 2
                    sdt = _F32 if hwdge else _BF16
                    slab = slabs.tile([P, 2, 2, nW, 2, D], sdt, tag="slab")
                    eng = nc.sync if hwdge else nc.gpsimd
                    eng.dma_start(
                        out=slab[:],
                        in_=inv4[:, t, :].rearrange(
                            "p (a two b) -> p a two b", a=2, two=2
                        ),
                    )
                    r = rpool.tile([P, 2, nW, 2, D], _BF16, tag="r")
                    nc.vector.tensor_add(
                        r[:], slab[:, :, 0, :, :, :], slab[:, :, 1, :, :, :]
                    )
                    nc.vector.tensor_add(
                        s_sg[:, ci : ci + 2, :, :], r[:, :, :, 0, :], r[:, :, :, 1, :]
                    )
                    ci += 2
            # square on ACT now: its input is ready and it hides under the
            # next SG's adds on DVE
            nsg = sg * nW
            sv = s_sg[:].rearrange("p s x d -> p (s x) d")
            sq2 = sqp.tile([P, nsg, D], _BF16, tag="sq2")
            nc.scalar.activation(sq2[:], sv, mybir.ActivationFunctionType.Square)
            return s_sg, sq2

        def emit_rest(sg, g0, s_sg, sq2):
            """tree-reduce over D + scale chain + final multiply + store."""
            nsg = sg * nW
            sv = s_sg[:].rearrange("p s x d -> p (s x) d")
            t8 = tree.tile([P, nsg, 8], _BF16, tag="t8")
            nc.vector.tensor_add(t8[:], sq2[:, :, 0:8], sq2[:, :, 8:16])
            t4 = tree.tile([P, nsg, 4], _BF16, tag="t4")
            nc.vector.tensor_add(t4[:], t8[:, :, 0:4], t8[:, :, 4:8])
            t2 = tree.tile([P, nsg, 2], _BF16, tag="t2")
            nc.vector.tensor_add(t2[:], t4[:, :, 0:2], t4[:, :, 2:4])
            ch = small.tile([P, nsg, 5], _F32, tag="ch")
            sq = ch[:, :, 0:1]
            c1 = ch[:, :, 1:2]
            a = ch[:, :, 2:3]
            rec = ch[:, :, 3:4]
            sc = ch[:, :, 4:5]
            nc.vector.tensor_add(sq, t2[:, :, 0:1], t2[:, :, 1:2])
            # scale = sqrt(sq) / (1 + sq)   (1e-8 dropped: sq >= O(1))
            nc.vector.tensor_scalar_add(c1, sq, 1.0)
            nc.scalar.activation(a, sq, mybir.ActivationFunctionType.Sqrt)
            nc.vector.reciprocal_approx_fast(rec, c1)
            nc.vector.tensor_mul(sc, a, rec)
            outt = outp.tile([P, sg, nW, D], _F32, tag="outt")
            ov = outt[:].rearrange("p s x d -> p (s x) d")
            nc.vector.tensor_mul(ov, sv, sc.to_broadcast((P, nsg, D)))
            nc.sync.dma_start(
                out=outv[:, g0 : g0 + sg, :],
                in_=ov.rearrange("p n d -> p (n d)"),
            )

        pending = []
        g0 = 0
        for si, sg in enumerate(sched):
            fine = si == 0 or si >= len(sched) - 2
            front = emit_front(sg, g0, fine=fine, hwdge=(si == 0))
            if pending:
                sg_p, g0_p, (s_p, sq_p) = pending.pop(0)
                emit_rest(sg_p, g0_p, s_p, sq_p)
            pending.append((sg, g0, front))
            g0 += sg
        for sg_p, g0_p, (s_p, sq_p) in pending:
            emit_rest(sg_p, g0_p, s_p, sq_p)


def build_nc(H=_H, W=_W, D=_D):
    """Build and compile the per-core Bass program."""
    nc = bacc.Bacc("TRN2", target_bir_lowering=False, debug=False)
    inp = nc.dram_tensor("inp", [128, H, W, D], _F32, kind="ExternalInput").ap()
    out = nc.dram_tensor(
        "out", [128, H // 2, W // 2, D], _F32, kind="ExternalOutput"
    ).ap()
    with tile.TileContext(nc) as tc:
        _kernel_body(tc, out, inp, H, W, D)
    nc.compile()
    return nc


_NC_CACHE = {}


def _get_nc():
    if "nc" not in _NC_CACHE:
        _NC_CACHE["nc"] = build_nc()
    return _NC_CACHE["nc"]


def kernel(inp, kernel_size=2, routing_iteration=3, _trace=False, _tmpdir=None):
    inp = np.asarray(inp, dtype=np.float32)
    assert int(kernel_size) == 2, "kernel compiled for kernel_size=2"
    assert inp.shape == (_B, _C, _H, _W, _D), inp.shape
    # routing_iteration is mathematically irrelevant (softmax over singleton
    # axis -> coefficients identically 1); any value >= 1 gives this output.

    nc = _get_nc()
    bpc = _B // _NCORES  # batches per core
    in_maps = [
        {"inp": np.ascontiguousarray(inp[i * bpc : (i + 1) * bpc]).reshape(128, _H, _W, _D)}
        for i in range(_NCORES)
    ]
    res = run_bass_kernel_spmd(
        nc, in_maps, core_ids=list(range(_NCORES)), trace=_trace, tmpdir=_tmpdir
    )
    out = np.empty((_B, _C, _H // 2, _W // 2, _D), dtype=np.float32)
    for i in range(_NCORES):
        out[i * bpc : (i + 1) * bpc] = res.results[i]["out"].reshape(
            bpc, _C, _H // 2, _W // 2, _D
        )
    if _trace:
        return out, res
    return out
